# revision 1
# baseline (speedup 1.0000x reference)
"""Fused CE + all-pairs cosine-embedding-loss kernel for Trainium2 (8 cores).

loss = CE(logits, labels) + 0.1 * mean_{i!=j} relu(cos(f_i, f_j))

Sharding: data-parallel over N=4096 rows (512 rows/core). Each core:
  - streams its logits shard [512, 32000] once from HBM, computing
    per-row sum(exp(x)) on the scalar engine (Exp + accum_out), then
    logZ = ln(S); the target logit is gathered with an indirect DMA.
  - computes its slice of the Gram matrix G = F_shard @ F_all^T on the
    tensor engine in bf16 (features^T resident in SBUF), applies relu
    while evacuating PSUM, and contracts rows with rinv_i via a second
    matmul, yielding u_j = sum_i rinv_i * relu(G_ij)  (uses
    relu(cos * n_i * n_j) = n_i * n_j * relu(cos), n > 0).
Host combines 8 partial outputs (O(N) work): ce mean, rinv = 1/sqrt(n2),
contrastive = (sum_j (sum_c u_cj) * rinv_j - N) / (N*(N-1)).
"""
import os
import sys

import numpy as np

for _p in ("/opt/trn_rl_repo",):
    if _p not in sys.path:
        sys.path.append(_p)

import concourse.bass as bass
import concourse.tile as tile
from concourse import mybir
from concourse.bass_utils import run_bass_kernel_spmd

F32 = mybir.dt.float32
BF16 = mybir.dt.bfloat16
I32 = mybir.dt.int32
NP_BF16 = mybir.dt.np(BF16)
AF = mybir.ActivationFunctionType

N_CORES = 8
N, C, D = 4096, 32000, 1024
P = 128                      # partitions
SHARD = N // N_CORES         # 512 rows per core
R = SHARD // P               # 4 row-chunks per core
FC = 4000                    # logits column chunk
CC = C // FC                 # 8 column chunks
KD = D // P                  # 8 contraction chunks
NJ = 512                     # gram column tile (one PSUM bank)
J = N // NJ                  # 8 gram column chunks
ALPHA = 0.1

_NC_CACHE = None
LAST_RESULT = None


def _split_excess_waits(nc, cap=1):
    """The walrus build here rejects instructions with >2 sync waits; hoist
    extras onto standalone EventSemaphore ops (same engine, just before)."""
    n = 0
    for fn in nc.m.functions:
        for blk in fn.blocks:
            out = []
            for inst in blk.instructions:
                si = inst.sync_info
                if si is not None and len(si.on_wait) > cap:
                    waits = list(si.on_wait)
                    extra, keep = waits[:-cap], waits[-cap:]
                    for i, w in enumerate(extra):
                        out.append(
                            mybir.InstEventSemaphore(
                                name=f"{inst.name}-wsplit{i}",
                                engine=inst.engine,
                                ins=[],
                                outs=[],
                                sync_info=mybir.SyncInfo(on_wait=[w], on_update=[]),
                            )
                        )
                        n += 1
                    si.on_wait = keep
                out.append(inst)
            blk.instructions = out
    return n


def _build():
    nc = bass.Bass("TRN2")
    lg = nc.dram_tensor("lg", [SHARD, C], F32, kind="ExternalInput")
    ft = nc.dram_tensor("ft", [D, N], BF16, kind="ExternalInput")
    ftsh = nc.dram_tensor("ftsh", [D, SHARD], BF16, kind="ExternalInput")
    fs = nc.dram_tensor("fs", [SHARD, D], F32, kind="ExternalInput")
    gidx = nc.dram_tensor("gidx", [P, R], I32, kind="ExternalInput")
    u_out = nc.dram_tensor("u_out", [1, N], F32, kind="ExternalOutput")
    n2_out = nc.dram_tensor("n2_out", [P, R], F32, kind="ExternalOutput")
    ce_out = nc.dram_tensor("ce_out", [P, R], F32, kind="ExternalOutput")

    with tile.TileContext(nc) as tc:
        with (
            tc.tile_pool(name="persist", bufs=1) as persist,
            tc.tile_pool(name="lgp", bufs=3) as lgp,
            tc.tile_pool(name="sqp", bufs=2) as sqp,
            tc.tile_pool(name="relup", bufs=3) as relup,
            tc.tile_pool(name="gpsum", bufs=3, space="PSUM") as gpsum,
            tc.tile_pool(name="upsum", bufs=2, space="PSUM") as upsum,
        ):
            # ---- resident loads ----
            ft_t = persist.tile([P, KD, N], BF16)
            nc.sync.dma_start(out=ft_t[:], in_=ft[:].rearrange("(k p) n -> p k n", p=P))
            ftsh_t = persist.tile([P, KD, SHARD], BF16)
            nc.sync.dma_start(
                out=ftsh_t[:], in_=ftsh[:].rearrange("(k p) m -> p k m", p=P)
            )
            fs_t = persist.tile([P, R, D], F32)
            nc.sync.dma_start(out=fs_t[:], in_=fs[:].rearrange("(r p) d -> p r d", p=P))
            gidx_t = persist.tile([P, R], I32)
            nc.sync.dma_start(out=gidx_t[:], in_=gidx[:])

            # ---- shard norms -> rinv ----
            n2_t = persist.tile([P, R], F32)
            for r in range(R):
                sq = sqp.tile([P, D], F32)
                nc.vector.tensor_mul(sq[:], fs_t[:, r], fs_t[:, r])
                nc.vector.tensor_reduce(
                    n2_t[:, r : r + 1], sq[:], axis=mybir.AxisListType.X,
                    op=mybir.AluOpType.add,
                )
            nc.sync.dma_start(out=n2_out[:], in_=n2_t[:])
            rstd = persist.tile([P, R], F32)
            nc.scalar.activation(out=rstd[:], in_=n2_t[:], func=AF.Sqrt)
            rinv = persist.tile([P, R], F32)
            nc.vector.reciprocal(out=rinv[:], in_=rstd[:])
            rinv_bf = persist.tile([P, R], BF16)
            nc.vector.tensor_copy(out=rinv_bf[:], in_=rinv[:])

            # ---- gather target logits ----
            tgt = persist.tile([P, R], F32)
            lg_flat = lg[:].rearrange("n c -> (n c)")[:, None]
            for r in range(R):
                nc.gpsimd.indirect_dma_start(
                    out=tgt[:, r : r + 1],
                    out_offset=None,
                    in_=lg_flat,
                    in_offset=bass.IndirectOffsetOnAxis(
                        ap=gidx_t[:, r : r + 1], axis=0
                    ),
                )

            # ---- gram / contrastive ----
            u_sb = persist.tile([1, N], F32)
            for j in range(J):
                up = upsum.tile([1, NJ], F32, space="PSUM")
                for r in range(R):
                    gp = gpsum.tile([P, NJ], F32, space="PSUM")
                    for k in range(KD):
                        nc.tensor.matmul(
                            out=gp[:],
                            lhsT=ftsh_t[:, k, r * P : (r + 1) * P],
                            rhs=ft_t[:, k, j * NJ : (j + 1) * NJ],
                            start=(k == 0),
                            stop=(k == KD - 1),
                        )
                    rt = relup.tile([P, NJ], BF16)
                    nc.vector.tensor_scalar_max(rt[:], gp[:], 0.0)
                    nc.tensor.matmul(
                        out=up[:],
                        lhsT=rinv_bf[:, r : r + 1],
                        rhs=rt[:],
                        start=(r == 0),
                        stop=(r == R - 1),
                    )
                nc.vector.tensor_copy(out=u_sb[:, j * NJ : (j + 1) * NJ], in_=up[:])
            nc.sync.dma_start(out=u_out[:], in_=u_sb[:])

            # ---- cross entropy: streaming sum(exp(x)) ----
            sexp = persist.tile([P, R, CC], F32)
            lg_v = lg[:].rearrange("(r p) c -> r p c", p=P)
            for r in range(R):
                for cc in range(CC):
                    t = lgp.tile([P, FC], F32)
                    nc.sync.dma_start(
                        out=t[:], in_=lg_v[r, :, cc * FC : (cc + 1) * FC]
                    )
                    nc.scalar.activation(
                        out=t[:], in_=t[:], func=AF.Exp,
                        accum_out=sexp[:, r, cc : cc + 1],
                    )
            s_t = persist.tile([P, R], F32)
            nc.vector.tensor_reduce(
                s_t[:], sexp[:], axis=mybir.AxisListType.X, op=mybir.AluOpType.add
            )
            nc.scalar.activation(out=s_t[:], in_=s_t[:], func=AF.Ln)
            ce_t = persist.tile([P, R], F32)
            nc.vector.tensor_tensor(
                out=ce_t[:], in0=s_t[:], in1=tgt[:], op=mybir.AluOpType.subtract
            )
            nc.sync.dma_start(out=ce_out[:], in_=ce_t[:])

    _split_excess_waits(nc)
    return nc


def kernel(logits, labels, features):
    global _NC_CACHE, LAST_RESULT
    logits = np.ascontiguousarray(np.asarray(logits), dtype=np.float32)
    labels = np.asarray(labels).astype(np.int64)
    features = np.ascontiguousarray(np.asarray(features), dtype=np.float32)

    if _NC_CACHE is None:
        _NC_CACHE = _build()
    nc = _NC_CACHE

    ft_full = np.ascontiguousarray(features.T.astype(NP_BF16))  # [D, N] bf16
    row_base = np.arange(SHARD, dtype=np.int64) * C

    in_maps = []
    for c in range(N_CORES):
        lo, hi = c * SHARD, (c + 1) * SHARD
        flat = (row_base + labels[lo:hi]).astype(np.int32)
        gidx = np.ascontiguousarray(flat.reshape(R, P).T)  # [P, R]
        in_maps.append(
            {
                "lg": logits[lo:hi],
                "ft": ft_full,
                "ftsh": np.ascontiguousarray(ft_full[:, lo:hi]),
                "fs": features[lo:hi],
                "gidx": gidx,
            }
        )

    res = run_bass_kernel_spmd(nc, in_maps, core_ids=list(range(N_CORES)))
    LAST_RESULT = res

    ce_sum = 0.0
    v = np.zeros(N, dtype=np.float64)
    n2 = np.zeros(N, dtype=np.float64)
    for c in range(N_CORES):
        out = res.results[c]
        ce_sum += np.asarray(out["ce_out"], dtype=np.float64).sum()
        v += np.asarray(out["u_out"], dtype=np.float64).reshape(N)
        # n2_out[p, r] holds row c*SHARD + r*P + p
        n2[c * SHARD : (c + 1) * SHARD] = (
            np.asarray(out["n2_out"], dtype=np.float64).T.reshape(SHARD)
        )

    ce = ce_sum / N
    rinv = 1.0 / np.sqrt(n2)
    contrast_sum = float(v @ rinv) - N  # remove diagonal (cos_ii = 1)
    contrastive = contrast_sum / (N * (N - 1))
    return np.float32(ce + ALPHA * contrastive)
